# revision 9
# baseline (speedup 1.0000x reference)
"""CvT attention block (depthwise conv QKV + MHA) on 8 Trainium2 NeuronCores,
data-parallel over batch.

v5 (head-pair packed attention + DVE-assisted softmax exp):
  - Phase A/B as v4: convs as diagonal-weight PE matmuls, projections bf16.
    KT zero-padded to 7x128 kv-tiles so every QK LDWEIGHTS is a 128-col FWL
    load; V-hat drops the ones-column (denominators via M=1 matmuls).
  - Phase C processes head PAIRS at l-chunks of 256: QK row-tiled
    (head h on PE rows 0-63, h+1 on 64-127, concurrent), AV col-tiled
    (head h -> PSUM partitions 0-63, h+1 -> 64-127, concurrent), so the
    128x128 array is fully used despite d=64 heads.
  - softmax exp is split across TWO engines: ACT exps S[:, 0:2048]
    (kv-tiles 0-3); a custom 2-instruction DVE op exps S[:, 2048:3584]
    (kv-tiles 4-6): p = deg-3 Taylor of e^(z*SCALE/64), then p^64 via 6
    squarings (~1e-4 poly err, bf16-out bound).
  - PSUM: one [128,3584] S tile (7 banks, ring via bufs=1) + one shared
    bank cycling AV-accum/denoms -> rb -> out-proj windows. 1-block
    software pipeline: block i emits QK(i), AV(i-1), rb/norm(i-2),
    out-proj/store pieces of the previous chunk, keeping the PE dense so
    the HAM clock gate stays at 8/8.
"""

import contextlib
import numpy as np
import ml_dtypes
from concourse import mybir
import concourse.bacc as bacc
import concourse.tile as tile
from concourse.bass_utils import run_bass_kernel_spmd
from concourse.dve_ops import DveOp, OPS, CUSTOM_DVE_SPECS, _SUB_OPCODE_FOR_NAME
from concourse.dve_spec import Spec, Src0, C0, C1, C2, One, sq, lower
from concourse.dve_uop import DveOpSpec

F32 = mybir.dt.float32
BF16 = mybir.dt.bfloat16
AFT = mybir.ActivationFunctionType

C = 384
T = 3136            # 56*56
TKV = 784           # 28*28
TKVP = 896          # 7*128 (zero-padded)
NH = 6
SCALE = C ** (-0.5)
EPS = 1e-5
XB = 3368           # 2 + 58*58 + 8 slack; data (r,c) at 2 + (1+r)*58 + 1+c

LS = 256
LC = [(i * LS, min(LS, T - i * LS)) for i in range((T + LS - 1) // LS)]  # 13
TT = [(j * 128, min(128, TKV - j * 128)) for j in range(7)]  # kv tiles
TAPS = [(t // 3 - 1, t % 3 - 1) for t in range(9)]

_CACHE = {}


# ---- custom DVE exp: exp(x*SCALE) = (deg3 e^(x*SCALE/64))^64 ---------------
def _register_dve(op_name, body, reference):
    spec = Spec(body=body, reference=reference)
    shas = {}
    for ver in ("v3", "v4"):
        uops = lower(spec, ver=ver)
        shas[ver] = DveOpSpec(name=op_name, opcode=0, uops=uops,
                              rd1_en=False).sha(ver)
    op = DveOp(op_name, spec, subdim=False, uops_sha=shas)
    if op_name not in _SUB_OPCODE_FOR_NAME:
        OPS.append(op)
        CUSTOM_DVE_SPECS[op_name] = spec
        _SUB_OPCODE_FOR_NAME[op_name] = max(_SUB_OPCODE_FOR_NAME.values()) + 1
        assert _SUB_OPCODE_FOR_NAME[op_name] < 0x20
    return op


_u = Src0 * C0
EXP_P1 = _register_dve(
    "ANT_EXP_P1", ((_u * C1 + One) * _u * C2 + One) * _u + One,
    lambda in0, in1, s0, s1, imm2: (
        ((in0 * s0 * s1 + 1.0) * (in0 * s0) * imm2 + 1.0) * (in0 * s0) + 1.0),
)
EXP_P2 = _register_dve(
    "ANT_EXP_P2", sq(sq(sq(sq(sq(sq(Src0)))))),
    lambda in0, in1, s0, s1, imm2: in0 ** 64,
)


def _emit(nc, tc, ctx, d, reps):
    pers = ctx.enter_context(tc.tile_pool(name="pers", bufs=1))

    wq = [pers.tile([128, C], BF16, tag=f"wq{i}", name=f"wq{i}") for i in range(3)]
    wk = [pers.tile([128, C], BF16, tag=f"wk{i}", name=f"wk{i}") for i in range(3)]
    wv = [pers.tile([128, C], BF16, tag=f"wv{i}", name=f"wv{i}") for i in range(3)]
    wpj = [pers.tile([128, C], BF16, tag=f"wpj{i}", name=f"wpj{i}")
           for i in range(3)]
    wd = [pers.tile([128, 27 * 128], BF16, tag=f"wd{i}", name=f"wd{i}")
          for i in range(3)]
    ind2 = pers.tile([2, 128], BF16, tag="ind2", name="ind2")
    ones = pers.tile([128, 1], BF16, tag="ones", name="ones")
    ones2 = pers.tile([128, 2], BF16, tag="ones2", name="ones2")
    wb = [pers.tile([128, 30], F32, tag=f"wb{i}", name=f"wb{i}")
          for i in range(3)]
    bpjW = pers.tile([128, 1024], F32, tag="bpjW", name="bpjW")
    QT = [pers.tile([128, T], BF16, tag=f"QT{i}", name=f"QT{i}") for i in range(3)]
    KT = [pers.tile([128, TKVP], BF16, tag=f"KT{i}", name=f"KT{i}")
          for i in range(3)]
    Vh = [pers.tile([128, C], BF16, tag=f"Vh{i}", name=f"Vh{i}")
          for i in range(7)]

    nc.sync.dma_start(wd[0][:], d["wd"][0])
    nc.sync.dma_start(wb[0][:], d["wb"][0])
    nc.vector.memset(ones[:], 1.0)
    nc.vector.memset(ones2[:, 0:1], 0.0)
    nc.vector.memset(ones2[:, 1:2], 1.0)
    for i in range(3):
        nc.vector.memset(KT[i][:, TKV:TKVP], 0.0)

    def _late_dmas():
        for i in range(1, 3):
            nc.sync.dma_start(wd[i][:], d["wd"][i])
            nc.sync.dma_start(wb[i][:], d["wb"][i])
        for i in range(3):
            nc.sync.dma_start(wq[i][:], d["wq"][i * 128:(i + 1) * 128, :])
            nc.sync.dma_start(wk[i][:], d["wk"][i * 128:(i + 1) * 128, :])
            nc.sync.dma_start(wv[i][:], d["wv"][i * 128:(i + 1) * 128, :])
            nc.sync.dma_start(wpj[i][:], d["wpj"][i * 128:(i + 1) * 128, :])
        nc.sync.dma_start(ind2[:], d["ind2"])
        nc.sync.dma_start(bpjW[:], d["bpjW"])

    for rep in range(reps):
        sfx = f"r{rep}"
        with contextlib.ExitStack() as phAB:
            ypool = phAB.enter_context(tc.tile_pool(name="y" + sfx, bufs=1))
            yq = [ypool.tile([128, T], BF16, tag=f"yq{i}", name=f"yq{i}")
                  for i in range(3)]
            yk = [ypool.tile([128, TKV], BF16, tag=f"yk{i}", name=f"yk{i}")
                  for i in range(3)]
            yv = [ypool.tile([128, TKV], BF16, tag=f"yv{i}", name=f"yv{i}")
                  for i in range(3)]
            psAB = phAB.enter_context(
                tc.tile_pool(name="psAB" + sfx, bufs=1, space="PSUM"))
            xpool = phAB.enter_context(tc.tile_pool(name="x" + sfx, bufs=2))

            # ---- Phase A: all convs on PE via diagonal-weight matmuls ----
            for ch in range(3):
                xb = xpool.tile([128, XB], BF16, tag="x", name="x")
                nc.sync.dma_start(xb[:], d["xb"][ch * 128:(ch + 1) * 128, :])
                if rep == 0 and ch == 0:
                    _late_dmas()
                x3 = xb[:, 2:2 + 3364].rearrange("p (r c) -> p r c", c=58)
                # q: stride 1, 7 chunks of 8 rows (464 padded cols each)
                for k in range(7):
                    pcv = psAB.tile([128, 512], F32, tag="pcv", name="pcv",
                                    bufs=4)
                    base = 2 + (1 + 8 * k) * 58
                    for t, (di, dj) in enumerate(TAPS):
                        nc.tensor.matmul(pcv[:, 0:464],
                                         wd[ch][:, t * 128:(t + 1) * 128],
                                         xb[:, base + 58 * di + dj:
                                            base + 58 * di + dj + 464],
                                         start=(t == 0), stop=(t == 8))
                    src = pcv[:, 0:464].rearrange(
                        "p (r c) -> p r c", c=58)[:, :, 1:57]
                    dst = yq[ch][:, 448 * k:448 * (k + 1)].rearrange(
                        "p (r c) -> p r c", c=56)
                    nc.vector.tensor_scalar_add(dst, src, wb[ch][:, 27:28])
                # k/v: stride 2, 2 chunks of 14 rows (392 cols each)
                for cv, ykv in ((1, yk[ch]), (2, yv[ch])):
                    for r0 in (0, 14):
                        pcv = psAB.tile([128, 512], F32, tag="pcv",
                                        name="pcv", bufs=4)
                        for t, (di, dj) in enumerate(TAPS):
                            mv = x3[:, 1 + 2 * r0 + di:1 + 2 * r0 + di + 28:2,
                                    1 + dj:1 + dj + 56:2]
                            nc.tensor.matmul(
                                pcv[:, 0:392],
                                wd[ch][:, (9 * cv + t) * 128:
                                       (9 * cv + t + 1) * 128],
                                mv, start=(t == 0), stop=(t == 8))
                        nc.vector.tensor_scalar_add(
                            ykv[:, r0 * 28:r0 * 28 + 392], pcv[:, 0:392],
                            wb[ch][:, 27 + cv:28 + cv])

            # ---- Phase B: projections (bf16, double-buffered 1024 PSUM) ----
            LC512 = [(i * 512, min(512, T - i * 512)) for i in range(7)]
            for co in range(3):
                for g in range(4):
                    grp = LC512[2 * g:2 * g + 2]
                    p = psAB.tile([128, 1024], F32, tag="psB", name="psB",
                                  bufs=2)
                    for k, (lo, ls) in enumerate(grp):
                        for ch in range(3):
                            nc.tensor.matmul(
                                p[0:128, k * 512:k * 512 + ls],
                                wq[ch][:, co * 128:(co + 1) * 128],
                                yq[ch][:, lo:lo + ls],
                                start=(ch == 0), stop=(ch == 2))
                    base = grp[0][0]
                    wid = grp[-1][0] + grp[-1][1] - base
                    nc.vector.tensor_copy(QT[co][:, base:base + wid],
                                          p[:, 0:wid])
                p = psAB.tile([128, 1024], F32, tag="psB", name="psB", bufs=2)
                for k, (to, ts) in enumerate(((0, 512), (512, 272))):
                    for ch in range(3):
                        nc.tensor.matmul(
                            p[:, k * 512:k * 512 + ts],
                            wk[ch][:, co * 128:(co + 1) * 128],
                            yk[ch][:, to:to + ts],
                            start=(ch == 0), stop=(ch == 2))
                nc.vector.tensor_copy(KT[co][:, 0:TKV], p[:, 0:TKV])
            for g in range(4):
                tt = TT[2 * g:2 * g + 2]
                p = psAB.tile([128, 1024], F32, tag="psB", name="psB", bufs=2)
                for k, (to, ts) in enumerate(tt):
                    for ch in range(3):
                        nc.tensor.matmul(
                            p[0:ts, k * 512:k * 512 + C],
                            yv[ch][:, to:to + ts], wv[ch][:],
                            start=(ch == 0), stop=(ch == 2))
                for k, (to, ts) in enumerate(tt):
                    ti = 2 * g + k
                    if ti >= 7:
                        continue
                    nc.vector.tensor_copy(
                        Vh[ti][0:ts, :], p[0:ts, k * 512:k * 512 + C])

        # ---- Phase C: head-pair attention, 1-block software pipeline ----
        with contextlib.ExitStack() as phC:
            cw = phC.enter_context(tc.tile_pool(name="cw" + sfx, bufs=2))
            psS = phC.enter_context(
                tc.tile_pool(name="psS" + sfx, bufs=1, space="PSUM"))

            def scol(j, ho):
                # ho-major S layout so the two concurrent row-tiled QK
                # matmuls of a head pair always drain to different banks
                return 256 * j + 1792 * ho

            def qk_block(c, p):
                """QK for pair p at chunk c: 14 row-tiled MMs + exp."""
                lo, ls = LC[c]
                S = psS.tile([128, 3584], F32, tag="S", name="S")
                for j in range(7):
                    for ho in range(2):
                        o = scol(j, ho)
                        nc.tensor.matmul(
                            S[0:128, o:o + ls],
                            KT[p][64 * ho:64 * ho + 64, 128 * j:128 * (j + 1)],
                            QT[p][64 * ho:64 * ho + 64, lo:lo + ls],
                            start=True, stop=True)
                etA = cw.tile([128, 2048], BF16, tag="etA", name="etA")
                nc.scalar.activation(etA[:], S[:, 0:2048], AFT.Exp,
                                     scale=float(SCALE))
                return {"c": c, "p": p, "S": S, "etA": etA}

            def exp_dve(st):
                tmp = cw.tile([128, 1536], F32, tag="tmp", name="tmp")
                etB = cw.tile([128, 1536], BF16, tag="etB", name="etB")
                with nc.allow_low_precision(reason="bf16 softmax exp"):
                    nc.vector._custom_dve(
                        EXP_P1, out=tmp[:], in0=st["S"][:, 2048:3584],
                        s0=float(SCALE / 64.0), s1=1.0 / 3.0, imm2=0.5)
                    nc.vector._custom_dve(EXP_P2, out=etB[:], in0=tmp[:])
                st["etB"] = etB

            def av_block(st, OTb, rcfs):
                """AV + denominators for pair st, writes OTb/rcf slices."""
                c, p = st["c"], st["p"]
                lo, ls = LC[c]
                av = psS.tile([128, 512], F32, tag="av", name="av")

                def et_slice(j, ho, ls):
                    o = scol(j, ho)
                    if o < 2048:
                        return st["etA"][0:TT[j][1], o:o + ls]
                    return st["etB"][0:TT[j][1], o - 2048:o - 2048 + ls]

                for ho in range(2):
                    h = 2 * p + ho
                    for j, (to, ts) in enumerate(TT):
                        nc.tensor.matmul(
                            av[64 * ho:64 * ho + 64, 0:ls],
                            Vh[j][0:ts, 64 * h:64 * h + 64],
                            et_slice(j, ho, ls),
                            start=(j == 0), stop=(j == 6))
                # denominators: head-odd first via (zeros|ones) M=2 weights so
                # its sum drains to partition 1; head-even then re-claims
                # partition 0 (start=True clears it element-wise) -> both
                # denominators land on contiguous partitions {0, 1}.
                for j, (to, ts) in enumerate(TT):
                    nc.tensor.matmul(
                        av[0:2, 256:256 + ls], ones2[0:ts, 0:2],
                        et_slice(j, 1, ls), start=(j == 0), stop=(j == 6))
                for j, (to, ts) in enumerate(TT):
                    nc.tensor.matmul(
                        av[0:1, 256:256 + ls], ones[0:ts, 0:1],
                        et_slice(j, 0, ls), start=(j == 0), stop=(j == 6))
                nc.vector.tensor_copy(OTb[:, 256 * p:256 * p + ls],
                                      av[:, 0:ls])
                rcf = cw.tile([2, 256], F32, tag="rcf", name="rcf", bufs=3)
                rcA = cw.tile([2, 256], F32, tag="rcA", name="rcA", bufs=3)
                rc2r = cw.tile([2, 256], BF16, tag="rc2r", name="rc2r", bufs=3)
                nc.vector.tensor_copy(rcf[0:2, 0:ls], av[0:2, 256:256 + ls])
                nc.vector.reciprocal_approx_fast(rcA[:, 0:ls], rcf[:, 0:ls])
                with nc.allow_low_precision(reason="bf16 softmax recip"):
                    nc.vector.tensor_copy(rc2r[:, 0:ls], rcA[:, 0:ls])
                st["rc2r"] = rc2r
                st["OTb"] = OTb

            def rb_norm(st):
                """Normalize pair st's OTb slice (deferred 2 blocks)."""
                c, p = st["c"], st["p"]
                lo, ls = LC[c]
                rb = psS.tile([128, 512], F32, tag="av", name="rb")
                nc.tensor.matmul(rb[:, 0:ls], ind2[:],
                                 st["rc2r"][0:2, 0:ls], start=True, stop=True)
                nc.vector.tensor_mul(st["OTb"][:, 256 * p:256 * p + ls],
                                     st["OTb"][:, 256 * p:256 * p + ls],
                                     rb[:, 0:ls])

            def chunk_out(ct, step):
                """Out-projection windows + store for a finished chunk."""
                c, OTb, osb = ct["c"], ct["OTb"], ct["osb"]
                lo, ls = LC[c]
                nlt = (ls + 127) // 128
                if step < 2:
                    k = step
                    if k >= nlt:
                        return
                    lsz = min(128, ls - 128 * k)
                    win = psS.tile([128, 512], F32, tag="av", name="win")
                    for ch in range(3):
                        nc.tensor.matmul(
                            win[0:lsz, 0:C],
                            OTb[:, 256 * ch + 128 * k:
                                256 * ch + 128 * k + lsz],
                            wpj[ch][:], start=(ch == 0), stop=(ch == 2))
                    nc.vector.tensor_add(osb[0:lsz, 512 * k:512 * k + C],
                                         win[0:lsz, 0:C],
                                         bpjW[0:lsz, 512 * k:512 * k + C])
                else:
                    nlt = (ls + 127) // 128
                    ov = osb[:].rearrange("p (w c) -> p w c",
                                          c=512)[:, 0:nlt, 0:C]
                    lsz = min(128, ls - (nlt - 1) * 128)
                    if ls < 128:
                        ov = ov[0:ls]
                    dst = d["out"][lo:lo + ls, :].rearrange(
                        "(w p) c -> p w c", p=min(128, ls))
                    nc.sync.dma_start(dst, ov)

            # block loop: 13 chunks x 3 pairs, software-pipelined
            blocks = [(c, p) for c in range(len(LC)) for p in range(3)]
            prev = None          # block i-1 state (awaiting AV)
            rb_q = []            # states awaiting rb+norm (lag 2)
            out_q = []           # chunk contexts awaiting out-proj
            OTb = None
            for c, p in blocks:
                if p == 0:
                    OTb = cw.tile([128, 768], BF16, tag="OTb", name="OTb")
                    rcfs = None
                st = qk_block(c, p)
                if prev is not None:
                    av_block(prev, prev["OTb_t"], None)
                exp_dve(st)
                st["OTb_t"] = OTb
                # deferred: one rb+norm (lag 2) and one chunk-out step
                if rb_q:
                    rb_norm(rb_q.pop(0))
                if out_q:
                    ct = out_q[0]
                    chunk_out(ct, ct["step"])
                    ct["step"] += 1
                    if ct["step"] > 2:
                        out_q.pop(0)
                if prev is not None:
                    rb_q.append(prev)
                    if prev["p"] == 2:
                        osb = cw.tile([128, 1024], F32, tag="osb", name="osb")
                        out_q.append({"c": prev["c"], "OTb": prev["OTb_t"],
                                      "osb": osb, "step": 0})
                prev = st

            # drain the pipeline
            av_block(prev, prev["OTb_t"], None)
            rb_q.append(prev)
            osb = cw.tile([128, 1024], F32, tag="osb", name="osb")
            out_q.append({"c": prev["c"], "OTb": prev["OTb_t"], "osb": osb,
                          "step": 0})
            for st in rb_q:
                rb_norm(st)
            for ct in out_q:
                for step in range(ct["step"], 3):
                    chunk_out(ct, step)


def _build(reps=1):
    if reps in _CACHE:
        return _CACHE[reps]
    nc = bacc.Bacc("TRN2", target_bir_lowering=False, debug=False)
    d = {
        "xb": nc.dram_tensor("xb", [C, XB], BF16, kind="ExternalInput").ap(),
        "wb": nc.dram_tensor("wb", [3, 128, 30], F32, kind="ExternalInput").ap(),
        "wd": nc.dram_tensor("wd", [3, 128, 27 * 128], BF16,
                             kind="ExternalInput").ap(),
        "wq": nc.dram_tensor("wq", [C, C], BF16, kind="ExternalInput").ap(),
        "wk": nc.dram_tensor("wk", [C, C], BF16, kind="ExternalInput").ap(),
        "wv": nc.dram_tensor("wv", [C, C], BF16, kind="ExternalInput").ap(),
        "wpj": nc.dram_tensor("wpj", [C, C], BF16, kind="ExternalInput").ap(),
        "ind2": nc.dram_tensor("ind2", [2, 128], BF16,
                               kind="ExternalInput").ap(),
        "bpjW": nc.dram_tensor("bpjW", [128, 1024], F32,
                               kind="ExternalInput").ap(),
        "out": nc.dram_tensor("out", [T, C], F32, kind="ExternalOutput").ap(),
    }
    with tile.TileContext(nc) as tc:
        with contextlib.ExitStack() as ctx:
            _emit(nc, tc, ctx, d, reps)
    nc.compile()
    _CACHE[reps] = nc
    return nc


def _bpjw(bproj):
    w = np.zeros((128, 1024), np.float32)
    for k in range(2):
        w[:, k * 512:k * 512 + C] = bproj[None, :]
    return w


def _host_prep(x, conv_q, conv_k, conv_v, bn_q, bn_k, bn_v, Wq, Wk, Wv,
               Wproj, bproj):
    bf = ml_dtypes.bfloat16
    B = x.shape[0]
    x = np.asarray(x, np.float32)
    # 58x58 zero-padded bf16 image: data (r,c) at col 2 + (1+r)*58 + 1+c
    xb = np.zeros((B, C, XB), bf)
    xi = np.ascontiguousarray(x.transpose(0, 2, 1)).reshape(B, C, 56, 56)
    xb3 = xb[:, :, 2:2 + 3364].reshape(B, C, 58, 58)
    xb3[:, :, 1:57, 1:57] = xi.astype(bf)

    wb = np.zeros((3, 128, 30), np.float32)
    whs = []
    for cv, (w, bn) in enumerate(((conv_q, bn_q), (conv_k, bn_k),
                                  (conv_v, bn_v))):
        g, b, m, v = [np.asarray(bn[i], np.float64) for i in range(4)]
        a = g / np.sqrt(v + EPS)
        bias = (b - m * a).astype(np.float32)
        wh = (np.asarray(w, np.float64).reshape(C, 9) * a[:, None]).astype(
            np.float32)
        whs.append(wh)
        for ch in range(3):
            wb[ch, :, 9 * cv:9 * cv + 9] = wh[ch * 128:(ch + 1) * 128]
            wb[ch, :, 27 + cv] = bias[ch * 128:(ch + 1) * 128]

    # diag-packed conv weights for the PE:
    # wd[ch][p, (9*cv+t)*128 + q] = delta_pq * wh_cv[ch*128+p, t]
    wd = np.zeros((3, 128, 27 * 128), np.float32)
    idx = np.arange(128)
    for ch in range(3):
        for cv in range(3):
            for t in range(9):
                wd[ch, idx, (9 * cv + t) * 128 + idx] = \
                    whs[cv][ch * 128 + idx, t]

    ind2 = np.zeros((2, 128), np.float32)
    ind2[0, 0:64] = 1.0
    ind2[1, 64:128] = 1.0

    return {
        "xb": xb,
        "wb": wb,
        "wd": wd.astype(bf),
        "wq": np.asarray(Wq, np.float32).astype(bf),
        "wk": np.asarray(Wk, np.float32).astype(bf),
        "wv": np.asarray(Wv, np.float32).astype(bf),
        "wpj": np.asarray(Wproj, np.float32).astype(bf),
        "ind2": ind2.astype(bf),
        "bpjW": _bpjw(np.asarray(bproj, np.float32)),
    }


def kernel(x, h, w, conv_q, conv_k, conv_v, bn_q, bn_k, bn_v, Wq, Wk, Wv,
           Wproj, bproj, _reps=1, _nc=None):
    B = x.shape[0]
    nc = _nc if _nc is not None else _build(_reps)
    hp = _host_prep(x, conv_q, conv_k, conv_v, bn_q, bn_k, bn_v, Wq, Wk, Wv,
                    Wproj, bproj)
    shared = {k: v for k, v in hp.items() if k != "xb"}
    in_maps = [dict(shared, xb=hp["xb"][b]) for b in range(B)]
    res = run_bass_kernel_spmd(nc, in_maps, core_ids=list(range(B)))
    out = np.stack([res.results[b]["out"] for b in range(B)], axis=0)
    return out.astype(np.float32)
